# revision 1
# baseline (speedup 1.0000x reference)
"""Trainium2 Bass kernel for nn_Loss4PixelReconstruction.

reference: recon = sum_k shift_k(image1) * filters[k]  (11x11 dynamic
per-pixel filter, shared across RGB), loss = mean(sqrt((recon-image2)^2+eps^2)).

Sharding: data-parallel over (N=4) x (H split in 2) -> 8 cores.
Each core: local Charbonnier partial sum; host sums the 8 scalars.

v1: bf16 compute on DVE (2x tensor_tensor mode), ACT does fp32->bf16
conversions + Charbonnier sqrt with fused row-sum accumulation.
Layout: partition = h (128 rows/core), free = (c, w). Per-tap (dy,dx)
multiply uses dy-shifted bf16 image copies (even- and odd-offset copies
keep the 4B alignment needed for the DVE 2x packed mode); products for
one dy-row (11 taps) are tree-added, then accumulated across dy.
"""

import sys

sys.path.insert(0, "/opt/trn_rl_repo")

import numpy as np

K = 11
PAD = 5
EPS = 1e-3
N, C, H, W = 4, 3, 256, 256
HSH = 128               # output rows per core
IMG_H = HSH + 2 * PAD   # 138 padded input rows per core
W_PAD = 268             # padded input cols (5 + 256 + 7)
CW = C * W

_CACHE = {}
LAST_RESULTS = None


def _build_nc():
    import concourse.tile as tile
    from concourse import bacc, mybir
    from concourse import bass_isa
    from contextlib import ExitStack

    f32 = mybir.dt.float32
    bf16 = mybir.dt.bfloat16
    MUL = mybir.AluOpType.mult
    ADD = mybir.AluOpType.add
    SUB = mybir.AluOpType.subtract

    nc = bacc.Bacc("TRN2", target_bir_lowering=False, debug=False)

    img1p = nc.declare_dram_parameter("img1p", [C, IMG_H, W_PAD], f32, isOutput=False)
    img2 = nc.declare_dram_parameter("img2", [C, HSH, W], f32, isOutput=False)
    flt = nc.declare_dram_parameter("flt", [K * K, HSH, W], f32, isOutput=False)
    out = nc.declare_dram_parameter("out", [1, 1], f32, isOutput=True)

    with ExitStack() as ctx:
        tc = ctx.enter_context(tile.TileContext(nc))
        stagep = ctx.enter_context(tc.tile_pool(name="stage", bufs=3))
        imp = ctx.enter_context(tc.tile_pool(name="im", bufs=1))
        ffpp = ctx.enter_context(tc.tile_pool(name="ffp", bufs=2))
        fbfp = ctx.enter_context(tc.tile_pool(name="fbf", bufs=2))
        prodp = ctx.enter_context(tc.tile_pool(name="prod", bufs=2))
        trep = ctx.enter_context(tc.tile_pool(name="tre", bufs=2))
        accp = ctx.enter_context(tc.tile_pool(name="acc", bufs=1))
        tailp = ctx.enter_context(tc.tile_pool(name="tail", bufs=1))

        # dy-shifted bf16 image copies; _o is shifted one more column so
        # odd-dx taps read at 4B-aligned offsets.
        ime = imp.tile([HSH, K, C, W_PAD], bf16)
        imo = imp.tile([HSH, K, C, W_PAD], bf16)
        for dy in range(K):
            st = stagep.tile([HSH, C, W_PAD], f32, tag="stage")
            nc.sync.dma_start(
                st[:], img1p[:, dy:dy + HSH, :].rearrange("c h w -> h c w")
            )
            nc.scalar.copy(ime[:, dy, :, :], st[:])
            nc.scalar.copy(imo[:, dy, :, 0:W_PAD - 2], st[:, :, 1:W_PAD - 1])

        dyacc = accp.tile([HSH, CW], bf16)

        for dy in range(K):
            ff = ffpp.tile([HSH, K, W], f32)
            nc.sync.dma_start(
                ff[:], flt[dy * K:(dy + 1) * K, :, :].rearrange("k h w -> h k w")
            )
            fb = fbfp.tile([HSH, K, W], bf16)
            nc.scalar.copy(fb[:], ff[:])

            pr = prodp.tile([HSH, K, CW], bf16)
            for dx in range(K):
                if dx % 2 == 0:
                    src = ime[:, dy, :, dx:dx + W]
                else:
                    src = imo[:, dy, :, dx - 1:dx - 1 + W]
                fbc = fb[:, dx:dx + 1, :].broadcast_to([HSH, C, W])
                dst = pr[:, dx, :].rearrange("p (c w) -> p c w", c=C)
                nc.vector.tensor_tensor(dst, src, fbc, MUL)

            # tree-reduce the 11 product planes
            pr10 = pr[:, 0:10, :].rearrange("p (t j) cw -> p t j cw", j=2)
            t1 = trep.tile([HSH, 5, CW], bf16, tag="t1")
            nc.vector.tensor_tensor(
                t1[:], pr10[:, :, 0, :], pr10[:, :, 1, :], ADD
            )
            t14 = t1[:, 0:4, :].rearrange("p (t j) cw -> p t j cw", j=2)
            t2 = trep.tile([HSH, 2, CW], bf16, tag="t2")
            nc.vector.tensor_tensor(t2[:], t14[:, :, 0, :], t14[:, :, 1, :], ADD)
            tA = trep.tile([HSH, CW], bf16, tag="tA")
            nc.vector.tensor_tensor(tA[:], t2[:, 0, :], t2[:, 1, :], ADD)
            tB = trep.tile([HSH, CW], bf16, tag="tB")
            nc.vector.tensor_tensor(tB[:], t1[:, 4, :], pr[:, 10, :], ADD)
            if dy == 0:
                nc.vector.tensor_tensor(dyacc[:], tA[:], tB[:], ADD)
            else:
                tC = trep.tile([HSH, CW], bf16, tag="tC")
                nc.vector.tensor_tensor(tC[:], tA[:], tB[:], ADD)
                nc.vector.tensor_tensor(dyacc[:], dyacc[:], tC[:], ADD)

        # Charbonnier tail
        i2s = tailp.tile([HSH, C, W], f32)
        nc.sync.dma_start(i2s[:], img2[:, :, :].rearrange("c h w -> h c w"))
        i2b = tailp.tile([HSH, CW], bf16)
        nc.scalar.copy(i2b[:], i2s[:].rearrange("p c w -> p (c w)"))
        diff = tailp.tile([HSH, CW], bf16)
        nc.vector.tensor_tensor(diff[:], dyacc[:], i2b[:], SUB)
        d2 = tailp.tile([HSH, CW], bf16)
        nc.vector.tensor_tensor(d2[:], diff[:], diff[:], MUL)
        charb = tailp.tile([HSH, CW], f32)
        rowsum = tailp.tile([HSH, 1], f32)
        eps2 = tailp.tile([HSH, 1], f32)
        nc.vector.memset(eps2[:], EPS * EPS)
        nc.scalar.activation(
            charb[:], d2[:], mybir.ActivationFunctionType.Sqrt,
            bias=eps2[:], scale=1.0, accum_out=rowsum[:],
        )
        total = tailp.tile([HSH, 1], f32)
        nc.gpsimd.partition_all_reduce(
            total[:], rowsum[:], channels=HSH, reduce_op=bass_isa.ReduceOp.add
        )
        nc.sync.dma_start(out[:, :], total[0:1, :])

    nc.compile()
    return nc


def _get_nc():
    if "nc" not in _CACHE:
        _CACHE["nc"] = _build_nc()
    return _CACHE["nc"]


def _shard_inputs(image1, image2, filters):
    in_maps = []
    for core in range(8):
        n, hb = core // 2, core % 2
        h0 = hb * HSH
        img1p = np.zeros((C, IMG_H, W_PAD), np.float32)
        lo = max(0, h0 - PAD)
        hi = min(H, h0 + HSH + PAD)
        img1p[:, lo - (h0 - PAD):lo - (h0 - PAD) + (hi - lo), PAD:PAD + W] = \
            image1[n, :, lo:hi, :]
        in_maps.append({
            "img1p": img1p,
            "img2": np.ascontiguousarray(image2[n, :, h0:h0 + HSH, :]),
            "flt": np.ascontiguousarray(filters[n, :, h0:h0 + HSH, :]),
        })
    return in_maps


def kernel(image1, image2, filters):
    global LAST_RESULTS
    import os
    from concourse.bass_utils import run_bass_kernel_spmd

    nc = _get_nc()
    in_maps = _shard_inputs(
        np.asarray(image1, np.float32),
        np.asarray(image2, np.float32),
        np.asarray(filters, np.float32),
    )
    trace = bool(int(os.environ.get("KERNEL_TRACE", "0")))
    res = run_bass_kernel_spmd(nc, in_maps, list(range(8)), trace=trace)
    LAST_RESULTS = res
    parts = [float(res.results[i]["out"][0, 0]) for i in range(8)]
    return np.float32(sum(parts) / (N * C * H * W))



# revision 5
# speedup vs baseline: 1.1070x; 1.1070x over previous
"""Trainium2 Bass kernel for nn_Loss4PixelReconstruction.

reference: recon = sum_k shift_k(image1) * filters[k]  (11x11 dynamic
per-pixel filter, shared across RGB), loss = mean(sqrt((recon-image2)^2+eps^2)).

Sharding: data-parallel over (N=4) x (H split in 2) -> 8 cores.
Each core: local Charbonnier partial sum; host sums the 8 scalars.

v2: host pre-converts all inputs to bf16 in the exact SBUF layouts
(halves filter DMA vs f32 and removes all on-device staging copies).
Per dy the DVE runs only 5 wide instructions: 2 batched muls (even /
odd dx taps via overlapping-window access patterns over pre-shifted
image copies), an in-place pair add, a 6->3 add, and a 3-plane
accumulate. Tail: DVE diff+square, ACT sqrt with row-sum accumulate,
PE matmul-with-ones for the cross-partition reduce.
"""

import sys

sys.path.insert(0, "/opt/trn_rl_repo")

import numpy as np

K = 11
PAD = 5
EPS = 1e-3
N, C, H, W = 4, 3, 256, 256
HSH = 128               # output rows per core
IMG_H = HSH + 2 * PAD   # 138 padded input rows per core
W_PAD = 268             # padded input cols (5 + 256 + 7)
CW = C * W              # 768
ROW = C * W_PAD         # 804 flat padded row

_CACHE = {}
LAST_RESULTS = None


def _win(base, off, dims):
    """Overlapping-window AP: keep base's partition dim, replace free dims
    with explicit [step, count] pairs (element units), offset in elements."""
    from concourse.ap import AP
    ap = [list(base.ap[0])] + [[int(s), int(n)] for s, n in dims]
    return AP(base.tensor, base.offset + off, ap)


def _build_nc():
    import concourse.tile as tile
    from concourse import bacc, mybir
    from concourse.bass import MemorySpace
    from contextlib import ExitStack

    f32 = mybir.dt.float32
    bf16 = mybir.dt.bfloat16
    MUL = mybir.AluOpType.mult
    ADD = mybir.AluOpType.add

    nc = bacc.Bacc("TRN2", target_bir_lowering=False, debug=False)

    # all bf16, host-prepared in SBUF layout
    ime_d = nc.declare_dram_parameter("ime", [K, HSH, ROW], bf16, isOutput=False)
    imo_d = nc.declare_dram_parameter("imo", [K, HSH, ROW], bf16, isOutput=False)
    flt_d = nc.declare_dram_parameter("flt", [K, HSH, K * W], bf16, isOutput=False)
    img2n_d = nc.declare_dram_parameter("img2n", [HSH, CW], bf16, isOutput=False)
    out = nc.declare_dram_parameter("out", [1, 1], f32, isOutput=True)

    with ExitStack() as ctx:
        tc = ctx.enter_context(tile.TileContext(nc))
        imep = ctx.enter_context(tc.tile_pool(name="ime", bufs=3))
        imop = ctx.enter_context(tc.tile_pool(name="imo", bufs=3))
        fltp = ctx.enter_context(tc.tile_pool(name="flt", bufs=3))
        prp = ctx.enter_context(tc.tile_pool(name="pr", bufs=1))
        accp = ctx.enter_context(tc.tile_pool(name="acc", bufs=1))
        tailp = ctx.enter_context(tc.tile_pool(name="tail", bufs=1))
        psump = ctx.enter_context(
            tc.tile_pool(name="ps", space=MemorySpace.PSUM, bufs=1)
        )

        # acc planes 0..2 accumulate taps; plane 3 holds -image2 so the
        # final 4->1 collapse directly yields recon - image2.
        acc = accp.tile([HSH, 4, CW], bf16)
        nc.sync.dma_start(acc[:, 3, :], img2n_d[:, :])

        eps2 = tailp.tile([HSH, 1], f32)
        ones = tailp.tile([HSH, 1], f32)
        dummy = tailp.tile([HSH, 1], f32)
        nc.vector.memset(eps2[:], EPS * EPS)
        nc.vector.memset(ones[:], 1.0)
        # pull the Sqrt ACT table load into the ramp
        nc.scalar.activation(
            dummy[:], eps2[:], mybir.ActivationFunctionType.Sqrt, bias=eps2[:]
        )

        pr = prp.tile([HSH, 6, CW], bf16)

        for dy in range(K):
            ie = imep.tile([HSH, ROW], bf16, tag="ie")
            nc.sync.dma_start(ie[:], ime_d[dy, :, :])
            io = imop.tile([HSH, ROW], bf16, tag="io")
            nc.sync.dma_start(io[:], imo_d[dy, :, :])
            fb = fltp.tile([HSH, K * W], bf16, tag="fb")
            nc.sync.dma_start(fb[:], flt_d[dy, :, :])

            # products for even dx taps {0,2,4,6,8,10} -> pr planes 0..5
            img_e = _win(ie[:], 0, [(2, 6), (W_PAD, C), (1, W)])
            flt_e = _win(fb[:], 0, [(2 * W, 6), (0, C), (1, W)])
            dst_e = pr[:, 0:6, :].rearrange("p x (c w) -> p x c w", c=C)
            nc.vector.tensor_tensor(dst_e, img_e, flt_e, MUL)

            # odd dx taps {1,3,5,7,9} multiply-accumulate is not possible;
            # compute into a scratch then pair-add. Scratch = planes 0..4 of
            # a second region: reuse pr by splitting: planes 0..5 even, and
            # write odd products added in two steps would need a buffer, so
            # use a dedicated odd tile.
            prod_o = prp.tile([HSH, 5, CW], bf16, tag="po")
            img_o = _win(io[:], 0, [(2, 5), (W_PAD, C), (1, W)])
            flt_o = _win(fb[:], W, [(2 * W, 5), (0, C), (1, W)])
            dst_o = prod_o[:].rearrange("p x (c w) -> p x c w", c=C)
            nc.vector.tensor_tensor(dst_o, img_o, flt_o, MUL)

            # L1: planes0..4 += odd products (plane5 = dx10 stays)
            nc.vector.tensor_tensor(
                pr[:, 0:5, :], pr[:, 0:5, :], prod_o[:], ADD
            )
            if dy == 0:
                # acc[0:3] = pr[0:3] + pr[3:6]
                nc.vector.tensor_tensor(
                    acc[:, 0:3, :], pr[:, 0:3, :], pr[:, 3:6, :], ADD
                )
            else:
                # L2: pr[0:3] += pr[3:6]
                nc.vector.tensor_tensor(
                    pr[:, 0:3, :], pr[:, 0:3, :], pr[:, 3:6, :], ADD
                )
                # L3: acc[0:3] += pr[0:3]
                nc.vector.tensor_tensor(
                    acc[:, 0:3, :], acc[:, 0:3, :], pr[:, 0:3, :], ADD
                )

        # tail: collapse 4 planes (3 partial sums + (-img2)) -> diff
        nc.vector.tensor_tensor(
            acc[:, 0:2, :], acc[:, 0:2, :], acc[:, 2:4, :], ADD
        )
        diff = tailp.tile([HSH, CW], bf16)
        nc.vector.tensor_tensor(diff[:], acc[:, 0, :], acc[:, 1, :], ADD)
        d2 = tailp.tile([HSH, CW], bf16)
        nc.vector.tensor_tensor(d2[:], diff[:], diff[:], MUL)
        charb = tailp.tile([HSH, CW], f32)
        rowsum = tailp.tile([HSH, 1], f32)
        nc.scalar.activation(
            charb[:], d2[:], mybir.ActivationFunctionType.Sqrt,
            bias=eps2[:], scale=1.0, accum_out=rowsum[:],
        )
        # cross-partition reduce on the PE: ones^T @ rowsum -> [1,1]
        psum = psump.tile([1, 1], f32)
        nc.tensor.matmul(psum[:], ones[:], rowsum[:], start=True, stop=True)
        total = tailp.tile([1, 1], f32)
        nc.scalar.copy(total[:], psum[:])
        nc.sync.dma_start(out[:, :], total[:])

    nc.compile()
    return nc


def _get_nc():
    if "nc" not in _CACHE:
        _CACHE["nc"] = _build_nc()
    return _CACHE["nc"]


def _shard_inputs(image1, image2, filters):
    import ml_dtypes

    bf16 = ml_dtypes.bfloat16
    in_maps = []
    for core in range(8):
        n, hb = core // 2, core % 2
        h0 = hb * HSH
        pad1 = np.zeros((C, IMG_H, W_PAD), np.float32)
        lo = max(0, h0 - PAD)
        hi = min(H, h0 + HSH + PAD)
        pad1[:, lo - (h0 - PAD):lo - (h0 - PAD) + (hi - lo), PAD:PAD + W] = \
            image1[n, :, lo:hi, :]
        pad1b = pad1.astype(bf16)
        # shifted-by-one-column copy for odd taps (4B alignment)
        pad1o = np.zeros_like(pad1b)
        pad1o[:, :, :W_PAD - 1] = pad1b[:, :, 1:]
        # [K, HSH, C*W_PAD]: dy-shifted row blocks in SBUF layout
        ime = np.stack([
            pad1b[:, dy:dy + HSH, :].transpose(1, 0, 2).reshape(HSH, ROW)
            for dy in range(K)
        ])
        imo = np.stack([
            pad1o[:, dy:dy + HSH, :].transpose(1, 0, 2).reshape(HSH, ROW)
            for dy in range(K)
        ])
        # [K(dy), HSH, K(dx)*W]
        flt = filters[n, :, h0:h0 + HSH, :].reshape(K, K, HSH, W) \
            .transpose(0, 2, 1, 3).reshape(K, HSH, K * W).astype(bf16)
        img2n = (-image2[n, :, h0:h0 + HSH, :]).transpose(1, 0, 2) \
            .reshape(HSH, CW).astype(bf16)
        in_maps.append({
            "ime": np.ascontiguousarray(ime),
            "imo": np.ascontiguousarray(imo),
            "flt": np.ascontiguousarray(flt),
            "img2n": np.ascontiguousarray(img2n),
        })
    return in_maps


def kernel(image1, image2, filters):
    global LAST_RESULTS
    import os
    from concourse.bass_utils import run_bass_kernel_spmd

    nc = _get_nc()
    in_maps = _shard_inputs(
        np.asarray(image1, np.float32),
        np.asarray(image2, np.float32),
        np.asarray(filters, np.float32),
    )
    trace = bool(int(os.environ.get("KERNEL_TRACE", "0")))
    res = run_bass_kernel_spmd(nc, in_maps, list(range(8)), trace=trace)
    LAST_RESULTS = res
    parts = [float(res.results[i]["out"][0, 0]) for i in range(8)]
    return np.float32(sum(parts) / (N * C * H * W))
